# revision 6
# baseline (speedup 1.0000x reference)
"""CantorAttention Trainium2 kernel.

Strategy
--------
8 cores = 2 (batch) x 4 (head-groups of 4 heads).  Each core:
  phase 1: qkv = x[b] @ W_qkv[:, my 768 cols] + b  (PE transposes x tiles on
           the fly; bias folded into the matmul as a K=1 accumulation row),
           Q rows -> DRAM [S, 256], K|V fused rows -> DRAM [S, 512].
  phase 2: queries are grouped host-side into clusters that share a key-union
           of <=128 keys (routes have ~255 distinct rows -> ~52 clusters).
           Per cluster: indirect-DMA row gathers of Q and K|V, PE transposes
           to head layout, scores = Qc @ KcT (per head), masked softmax
           (mask add on DVE, exp + row-sum on ACT, normalization folded into
           a second exp via a -ln(sum) bias), attn^T via PE transpose, then
           out^T = Vc^T @ attn^T, accumulated into X2^T SBUF tiles.
           Out-projection per cluster tile against W_out rows of my 256 dims.
Host sums the 4 partial outputs per batch and adds b_out.
"""

import os
import sys

import numpy as np

for _p in ("/opt/trn_rl_repo",):
    if os.path.isdir(_p) and _p not in sys.path:
        sys.path.insert(0, _p)

import concourse.bacc as bacc
import concourse.bass as bass
import concourse.mybir as mybir
import concourse.tile as tile
from concourse.bass_utils import run_bass_kernel_spmd
from concourse.masks import make_identity

B, S, DIM = 2, 4096, 1024
H, HD, KN = 16, 64, 64
SCALE = 1.0 / np.sqrt(HD).item()
QMAX = 128  # query slots per cluster
UMAX = 128  # max distinct keys per cluster
NCORES = 8
MASKVAL = -1.0e5
F32 = mybir.dt.float32
I32 = mybir.dt.int32


# ---------------------------------------------------------------- host planning
def _plan_clusters(routes: np.ndarray):
    """Group queries by identical route rows, chain-order groups by key-set
    overlap, and greedily pack them into clusters of <=QMAX queries whose key
    union stays <=UMAX.  Returns (qidx [QMAX,NC], kidx [QMAX,NC], nvalid [NC],
    mask [NC,QMAX,UMAX] f32, q_order [S])."""
    uniq, inv = np.unique(routes, axis=0, return_inverse=True)
    G = len(uniq)

    # greedy max-overlap chain over the distinct rows
    member = np.zeros((G, S), dtype=np.int8)
    for g in range(G):
        member[g, uniq[g]] = 1
    ov = member @ member.T
    order = [0]
    used = np.zeros(G, dtype=bool)
    used[0] = True
    for _ in range(G - 1):
        cand = np.where(~used)[0]
        nxt = int(cand[np.argmax(ov[order[-1], cand])])
        order.append(nxt)
        used[nxt] = True

    items = []  # (key_set, query_list)
    for g in order:
        qs = np.nonzero(inv == g)[0].tolist()
        ks = set(uniq[g].tolist())
        while len(qs) > QMAX:
            items.append((ks, qs[:QMAX]))
            qs = qs[QMAX:]
        if qs:
            items.append((ks, qs))

    clusters = []
    curq: list[int] = []
    curk: set[int] = set()
    for ks, qs in items:
        if len(curq) + len(qs) > QMAX or len(curk | ks) > UMAX:
            clusters.append((curq, sorted(curk)))
            curq, curk = [], set()
        curq = curq + qs
        curk = curk | ks
    if curq:
        clusters.append((curq, sorted(curk)))

    NC = len(clusters)
    qidx = np.zeros((QMAX, NC), dtype=np.int32)
    kidx = np.zeros((QMAX, NC), dtype=np.int32)
    nvalid = np.zeros(NC, dtype=np.int32)
    mask = np.zeros((NC, QMAX, UMAX), dtype=np.float32)
    q_order = []
    for i, (qs, ks) in enumerate(clusters):
        nv, u = len(qs), len(ks)
        nvalid[i] = nv
        qidx[:nv, i] = qs
        kidx[:u, i] = ks
        q_order.extend(qs)
        # mask[i, r, j] = 0 where key ks[j] is in routes[qs[r]], else MASKVAL.
        # Padded query rows stay all-zero (finite garbage, rows never stored).
        ks_arr = np.asarray(ks, dtype=np.int32)
        hit = (routes[np.asarray(qs)][:, :, None] == ks_arr[None, None, :]).any(axis=1)
        m = np.where(hit, 0.0, MASKVAL).astype(np.float32)
        mask[i, :nv, :u] = m
        mask[i, :nv, u:] = MASKVAL
    q_order = np.asarray(q_order, dtype=np.int64)
    assert len(q_order) == S and len(set(q_order.tolist())) == S
    return qidx, kidx, nvalid, mask, q_order


# ---------------------------------------------------------------- device kernel
def _build(NC: int, nvalid: np.ndarray):
    nc = bacc.Bacc("TRN2", target_bir_lowering=False, debug=False, num_devices=NCORES)
    Exp, Ln = mybir.ActivationFunctionType.Exp, mybir.ActivationFunctionType.Ln
    add = mybir.AluOpType.add

    xb = nc.dram_tensor("xb", [S, DIM], F32, kind="ExternalInput")
    wqkv = nc.dram_tensor("wqkv", [DIM, 768], F32, kind="ExternalInput")
    bqkv = nc.dram_tensor("bqkv", [1, 768], F32, kind="ExternalInput")
    wout = nc.dram_tensor("wout", [256, DIM], F32, kind="ExternalInput")
    qidx = nc.dram_tensor("qidx", [QMAX, NC], I32, kind="ExternalInput")
    kidx = nc.dram_tensor("kidx", [QMAX, NC], I32, kind="ExternalInput")
    maskd = nc.dram_tensor("maskd", [NC, QMAX, UMAX], F32, kind="ExternalInput")
    yp = nc.dram_tensor("yp", [S, DIM], F32, kind="ExternalOutput")

    row0 = np.concatenate([[0], np.cumsum(nvalid)]).astype(int)
    VARIANT = int(os.environ.get("KV", "4"))
    SUBV = os.environ.get("SUBV", "z")

    with tile.TileContext(nc) as tc:
        with (
            tc.tile_pool(name="const", bufs=1) as cp,
            tc.tile_pool(name="dram", bufs=1, space="DRAM") as dp,
        ):
            qn = dp.tile([S, 256], F32)
            kvn = dp.tile([S, 512], F32)

            id128 = cp.tile([128, 128], F32, tag="id128")
            make_identity(nc, id128[:])
            ones = cp.tile([1, 128], F32, tag="ones")
            nc.gpsimd.memset(ones[:], 1.0)
            bias_sb = cp.tile([1, 768], F32, tag="bias")
            nc.sync.dma_start(bias_sb[:], bqkv[:])
            w_sb = []
            for k in range(8):
                w = cp.tile([128, 768], F32, tag=f"w{k}")
                nc.sync.dma_start(w[:], wqkv[k * 128 : (k + 1) * 128, :])
                w_sb.append(w)
            wo_sb = []
            for t in range(2):
                w = cp.tile([128, DIM], F32, tag=f"wo{t}")
                nc.sync.dma_start(w[:], wout[t * 128 : (t + 1) * 128, :])
                wo_sb.append(w)
            qidx_sb = cp.tile([QMAX, NC], I32, tag="qidx")
            nc.sync.dma_start(qidx_sb[:], qidx[:])
            kidx_sb = cp.tile([QMAX, NC], I32, tag="kidx")
            nc.sync.dma_start(kidx_sb[:], kidx[:])
            x2t0 = cp.tile([128, NC * 128], F32, tag="x2t0")
            x2t1 = cp.tile([128, NC * 128], F32, tag="x2t1")

            # ---------------- phase 1: qkv projection ----------------
            with (
                tc.tile_pool(name="p1", bufs=2) as p1,
                tc.tile_pool(name="p1t", bufs=2, space="PSUM") as p1t,
                tc.tile_pool(name="p1m", bufs=2, space="PSUM") as p1m,
            ):
                for st in range(S // 128):
                    xn = p1.tile([128, DIM], F32, tag="xn")
                    nc.sync.dma_start(xn[:], xb[st * 128 : (st + 1) * 128, :])
                    xt = p1.tile([128, DIM], F32, tag="xt")  # x^T, kc-major
                    for half in range(2):
                        pt = p1t.tile([128, 512], F32, tag="pt")
                        for q in range(4):
                            kc = half * 4 + q
                            nc.tensor.transpose(
                                pt[:, q * 128 : (q + 1) * 128],
                                xn[:, kc * 128 : (kc + 1) * 128],
                                id128[:],
                            )
                        nc.vector.tensor_copy(
                            xt[:, half * 512 : (half + 1) * 512], pt[:]
                        )
                    qkv = p1.tile([128, 768], F32, tag="qkv")
                    for half in range(2):
                        ps = p1m.tile([128, 384], F32, tag="ps")
                        for kc in range(8):
                            nc.tensor.matmul(
                                ps[:],
                                lhsT=xt[:, kc * 128 : (kc + 1) * 128],
                                rhs=w_sb[kc][:, half * 384 : (half + 1) * 384],
                                start=(kc == 0),
                                stop=False,
                            )
                        nc.tensor.matmul(
                            ps[:],
                            lhsT=ones[:],
                            rhs=bias_sb[:, half * 384 : (half + 1) * 384],
                            start=False,
                            stop=True,
                        )
                        nc.vector.tensor_copy(
                            qkv[:, half * 384 : (half + 1) * 384], ps[:]
                        )
                    nc.sync.dma_start(
                        qn[st * 128 : (st + 1) * 128, :], qkv[:, 0:256]
                    )
                    nc.sync.dma_start(
                        kvn[st * 128 : (st + 1) * 128, :], qkv[:, 256:768]
                    )

            # ---------------- phase 2: clustered attention + out-proj --------
            with (
                tc.tile_pool(name="p2", bufs=2) as p2,
                tc.tile_pool(name="psqk", bufs=2, space="PSUM") as psqk,
                tc.tile_pool(name="pssa", bufs=1, space="PSUM") as pssa,
                tc.tile_pool(name="pso", bufs=2, space="PSUM") as pso,
                tc.tile_pool(name="psy", bufs=2, space="PSUM") as psy,
            ):
                for i in range(NC if VARIANT >= 2 else 0):
                    qg = p2.tile([128, 256], F32, tag="qg")
                    nc.gpsimd.indirect_dma_start(
                        out=qg[:],
                        out_offset=None,
                        in_=qn[:],
                        in_offset=bass.IndirectOffsetOnAxis(
                            ap=qidx_sb[:, i : i + 1], axis=0
                        ),
                    )
                    kvg = p2.tile([128, 512], F32, tag="kvg")
                    nc.gpsimd.indirect_dma_start(
                        out=kvg[:],
                        out_offset=None,
                        in_=kvn[:],
                        in_offset=bass.IndirectOffsetOnAxis(
                            ap=kidx_sb[:, i : i + 1], axis=0
                        ),
                    )
                    mt = p2.tile([128, UMAX], F32, tag="mt")
                    nc.sync.dma_start(mt[:], maskd[i])
                    if SUBV < "b":
                        continue

                    ptq = psqk.tile([64, 512], F32, tag="ptqk")
                    for h in range(4):
                        nc.tensor.transpose(
                            ptq[:, h * 128 : (h + 1) * 128],
                            qg[:, h * 64 : (h + 1) * 64],
                            id128[:],
                        )
                    qT = p2.tile([64, 512], F32, tag="qT")
                    nc.vector.tensor_copy(qT[:], ptq[:])

                    ptk = psqk.tile([64, 512], F32, tag="ptqk")
                    for h in range(4):
                        nc.tensor.transpose(
                            ptk[:, h * 128 : (h + 1) * 128],
                            kvg[:, h * 64 : (h + 1) * 64],
                            id128[:],
                        )
                    kT = p2.tile([64, 512], F32, tag="kT")
                    nc.vector.tensor_copy(kT[:], ptk[:])
                    if SUBV < "c":
                        continue

                    ps_s = pssa.tile([128, 512], F32, tag="ps_s")
                    for h in range(4):
                        nc.tensor.matmul(
                            ps_s[:, h * 128 : (h + 1) * 128],
                            lhsT=qT[:, h * 128 : (h + 1) * 128],
                            rhs=kT[:, h * 128 : (h + 1) * 128],
                            start=True,
                            stop=True,
                        )
                    if SUBV < "d":
                        continue
                    ms = p2.tile([128, 512], F32, tag="ms")
                    for h in range(4):
                        nc.vector.tensor_tensor(
                            out=ms[:, h * 128 : (h + 1) * 128],
                            in0=ps_s[:, h * 128 : (h + 1) * 128],
                            in1=mt[:],
                            op=add,
                        )
                    if SUBV < "e":
                        continue
                    sums = p2.tile([128, 4], F32, tag="sums")
                    scr = p2.tile([128, 512], F32, tag="scr")
                    for h in range(4):
                        nc.scalar.activation(
                            scr[:, h * 128 : (h + 1) * 128],
                            ms[:, h * 128 : (h + 1) * 128],
                            Exp,
                            scale=SCALE,
                            accum_out=sums[:, h : h + 1],
                        )
                    if SUBV < "f":
                        continue
                    nls = p2.tile([128, 4], F32, tag="nls")
                    nc.scalar.activation(nls[:], sums[:], Ln)
                    nc.vector.tensor_scalar_mul(nls[:], nls[:], -1.0)
                    att = p2.tile([128, 512], F32, tag="att")
                    for h in range(4):
                        nc.scalar.activation(
                            att[:, h * 128 : (h + 1) * 128],
                            ms[:, h * 128 : (h + 1) * 128],
                            Exp,
                            scale=SCALE,
                            bias=nls[:, h : h + 1],
                        )
                    if VARIANT < 3:
                        continue
                    ps_a = pssa.tile([128, 512], F32, tag="ps_a")
                    for h in range(4):
                        nc.tensor.transpose(
                            ps_a[:, h * 128 : (h + 1) * 128],
                            att[:, h * 128 : (h + 1) * 128],
                            id128[:],
                        )
                    aT = p2.tile([128, 512], F32, tag="aT")
                    nc.vector.tensor_copy(aT[:], ps_a[:])

                    ps_o = pso.tile([128, 256], F32, tag="ps_o")
                    for h in range(4):
                        c, r = h // 2, (h % 2) * 64
                        nc.tensor.matmul(
                            ps_o[r : r + 64, c * 128 : (c + 1) * 128],
                            lhsT=kvg[:, 256 + h * 64 : 256 + (h + 1) * 64],
                            rhs=aT[:, h * 128 : (h + 1) * 128],
                            start=True,
                            stop=True,
                        )
                    nc.vector.tensor_copy(
                        x2t0[:, i * 128 : (i + 1) * 128], ps_o[:, 0:128]
                    )
                    nc.vector.tensor_copy(
                        x2t1[:, i * 128 : (i + 1) * 128], ps_o[:, 128:256]
                    )

                    if VARIANT < 4:
                        continue
                    yb = p2.tile([128, DIM], F32, tag="yb")
                    for half in range(2):
                        ps_y = psy.tile([128, 512], F32, tag="ps_y")
                        nc.tensor.matmul(
                            ps_y[:],
                            lhsT=x2t0[:, i * 128 : (i + 1) * 128],
                            rhs=wo_sb[0][:, half * 512 : (half + 1) * 512],
                            start=True,
                            stop=False,
                        )
                        nc.tensor.matmul(
                            ps_y[:],
                            lhsT=x2t1[:, i * 128 : (i + 1) * 128],
                            rhs=wo_sb[1][:, half * 512 : (half + 1) * 512],
                            start=False,
                            stop=True,
                        )
                        nc.vector.tensor_copy(
                            yb[:, half * 512 : (half + 1) * 512], ps_y[:]
                        )
                    nv, r0 = int(nvalid[i]), int(row0[i])
                    nc.sync.dma_start(yp[r0 : r0 + nv, :], yb[0:nv, :])
    nc.compile()
    return nc


_BUILD_CACHE: dict = {}


def _make_in_maps(inputs):
    x = np.asarray(inputs["x"], dtype=np.float32)
    W_qkv = np.asarray(inputs["W_qkv"], dtype=np.float32)
    b_qkv = np.asarray(inputs["b_qkv"], dtype=np.float32)
    W_out = np.asarray(inputs["W_out"], dtype=np.float32)
    routes = np.asarray(inputs["routes"], dtype=np.int32)
    qidx, kidx, nvalid, mask, q_order = _plan_clusters(routes)
    in_maps = []
    for c in range(NCORES):
        b, hg = c // 4, c % 4
        wq = np.concatenate(
            [
                W_qkv[:, hg * 256 : (hg + 1) * 256],
                W_qkv[:, DIM + hg * 256 : DIM + (hg + 1) * 256],
                W_qkv[:, 2 * DIM + hg * 256 : 2 * DIM + (hg + 1) * 256],
            ],
            axis=1,
        ).copy()
        bq = np.concatenate(
            [
                b_qkv[hg * 256 : (hg + 1) * 256],
                b_qkv[DIM + hg * 256 : DIM + (hg + 1) * 256],
                b_qkv[2 * DIM + hg * 256 : 2 * DIM + (hg + 1) * 256],
            ]
        ).reshape(1, 768).copy()
        in_maps.append(
            {
                "xb": np.ascontiguousarray(x[b]),
                "wqkv": wq,
                "bqkv": bq,
                "wout": np.ascontiguousarray(W_out[hg * 256 : (hg + 1) * 256, :]),
                "qidx": qidx,
                "kidx": kidx,
                "maskd": mask,
            }
        )
    return in_maps


def kernel(x, W_qkv, b_qkv, W_out, b_out, routes):
    b_out = np.asarray(b_out, dtype=np.float32)
    routes = np.asarray(routes, dtype=np.int32)

    qidx, kidx, nvalid, mask, q_order = _plan_clusters(routes)
    NC = qidx.shape[1]

    key = (NC, nvalid.tobytes())
    if key not in _BUILD_CACHE:
        _BUILD_CACHE[key] = _build(NC, nvalid)
    nc = _BUILD_CACHE[key]

    in_maps = _make_in_maps(
        {"x": x, "W_qkv": W_qkv, "b_qkv": b_qkv, "W_out": W_out, "routes": routes}
    )

    res = run_bass_kernel_spmd(nc, in_maps, list(range(NCORES)))

    y = np.empty((B, S, DIM), dtype=np.float32)
    for b in range(B):
        acc = np.zeros((S, DIM), dtype=np.float32)
        for g in range(4):
            acc += res.results[b * 4 + g]["yp"]
        yb = np.empty((S, DIM), dtype=np.float32)
        yb[q_order] = acc
        y[b] = yb + b_out[None, :]
    return y
